# revision 1
# baseline (speedup 1.0000x reference)
"""GPTNet attention block — data-parallel over batch N across 8 NeuronCores.

Strategy (per sharding hint): pure data parallel over N=128 -> 16 samples/core.
All attention / 1x1 convs are per-sample; BatchNorm batch stats are computed
with a cross-device all-reduce (lax.pmean of per-device sum and sum-of-squares),
which reproduces the reference's global (N,T,V) training-mode statistics.
"""
import numpy as np
import jax
import jax.numpy as jnp

S, ST, CI = 3, 2, 16
NEG_SLOPE = 0.1
EPS = 1e-5
N_CORES = 8
AXIS = "dp"


def _conv1x1(x, W, b):
    return jnp.einsum('nctv,oc->notv', x, W) + b[None, :, None, None]


def _bn_dist(x, gamma, beta):
    # global training-mode BN over (N,T,V): all-reduce mean and mean-of-squares
    m1 = jax.lax.pmean(jnp.mean(x, axis=(0, 2, 3)), AXIS)
    m2 = jax.lax.pmean(jnp.mean(x * x, axis=(0, 2, 3)), AXIS)
    var = m2 - m1 * m1
    mu = m1[None, :, None, None]
    rstd = jax.lax.rsqrt(var + EPS)[None, :, None, None]
    return gamma[None, :, None, None] * (x - mu) * rstd + beta[None, :, None, None]


def _leaky(x):
    return jnp.where(x > 0, x, NEG_SLOPE * x)


def _forward_shard(x, p):
    N, C, T, V = x.shape
    qk = _conv1x1(x, p['W_qk_s'], p['b_qk_s']).reshape(N, 2 * S, CI, T, V)
    q, k = qk[:, :S], qk[:, S:]
    att = p['attention0s'][:, :, None] + jnp.tanh(
        jnp.einsum('nsctu,nsctv->nstuv', q, k) / CI) * p['alphas'][:, :, None]
    y = jnp.einsum('nctu,nstuv->nsctv', x, att).reshape(N, S * C, T, V)
    y = _bn_dist(_conv1x1(y, p['W_outs'], p['b_outs']), p['g_outs'], p['be_outs'])
    y = _leaky(x + y)
    y = _bn_dist(_conv1x1(y, p['W_ffs'], p['b_ffs']), p['g_ffs'], p['be_ffs'])
    s_out = _leaky(x + y)

    t_in = s_out
    Ct = t_in.shape[1]
    qk_t = _conv1x1(t_in, p['W_qk_t'], p['b_qk_t']).reshape(N, 4 * ST, CI, T, V).mean(-1)
    q_f, q_b = qk_t[:, :ST], qk_t[:, ST:2 * ST]
    k_f, k_b = qk_t[:, 2 * ST:3 * ST], qk_t[:, 3 * ST:]
    bmask = jnp.triu(jnp.ones((T, T), x.dtype))
    fmask = bmask.T
    att_b = jnp.tanh(jnp.einsum('nsct,nscq->nstq', q_b, k_b) / CI) * p['alphat_b'] * bmask
    att_f = jnp.tanh(jnp.einsum('nsct,nscq->nstq', q_f, k_f) / CI) * p['alphat_f'] * fmask
    z_f = jnp.einsum('nctv,nstq->nscqv', t_in, att_f).reshape(N, ST * Ct, T, V)
    z_b = jnp.einsum('nctv,nstq->nscqv', t_in, att_b).reshape(N, ST * Ct, T, V)
    z = jnp.concatenate([z_f, z_b], axis=1)
    z = _bn_dist(_conv1x1(z, p['W_outt'], p['b_outt']), p['g_outt'], p['be_outt'])
    z = _leaky(t_in + z)
    z = _bn_dist(_conv1x1(z, p['W_fft'], p['b_fft']), p['g_fft'], p['be_fft'])
    z = _leaky(t_in + z)

    z_tcn = jax.lax.conv_general_dilated(z, p['W_tcn'], (1, 1), ((3, 3), (0, 0)),
                                         dimension_numbers=('NCHW', 'OIHW', 'NCHW'))
    z_tcn = _bn_dist(z_tcn + p['b_tcn'][None, :, None, None], p['g_tcn'], p['be_tcn'])
    return _leaky(z + z_tcn)


_pmapped = None


def _get_pmapped():
    global _pmapped
    if _pmapped is None:
        _pmapped = jax.pmap(_forward_shard, axis_name=AXIS,
                            in_axes=(0, None), devices=jax.devices()[:N_CORES])
    return _pmapped


def kernel(**inputs) -> np.ndarray:
    x = np.asarray(inputs['x'], dtype=np.float32)
    p = {k: jnp.asarray(np.asarray(v, dtype=np.float32))
         for k, v in inputs.items() if k != 'x'}
    N = x.shape[0]
    per = N // N_CORES
    xs = jnp.asarray(x.reshape(N_CORES, per, *x.shape[1:]))
    try:
        out = _get_pmapped()(xs, p)
        out = np.asarray(out, dtype=np.float32).reshape(N, *out.shape[2:])
    except Exception:
        # fallback: single-device jit (still on NeuronCore 0), exact global BN
        def fwd1(x, p):
            return _forward_shard(x, p)
        one = jax.pmap(_forward_shard, axis_name=AXIS, in_axes=(0, None),
                       devices=jax.devices()[:1])
        out = one(jnp.asarray(x[None]), p)
        out = np.asarray(out, dtype=np.float32)[0]
    return out.astype(np.float32)

